# revision 13
# baseline (speedup 1.0000x reference)
"""VQ codebook nearest-neighbor kernel for Trainium2 (8 NeuronCores, SPMD).

v5: 2-pass fp16 matmuls + in-PSUM payload; no elementwise payload plumbing.
Super-groups of 4 tiles share a 2-bank PSUM tile; the payload pass is a
shape-uniform [128,128]x[128,1024] matmul (row 127 = ones x pay row, other
moving rows zero) so the PE never reconfigures tile size; stage-1 reduce is
batched over 4 tiles; eq/bs of level-2 run on GpSimd.

Problem: z [16, 64, 128, 128] f32, emb [256, 64] f32 ->
         codes [16, 128, 128] int32 = argmin_k ||x_p - emb_k||_2
         (x = z rearranged 't a b c -> t (b c) a').

Per 128-point tile (fp16 splits: z = zh + zm, E = -2 emb^T = Eh + Em):
  P1 [68 rows]:  zh.Eh + (|e|^2 + B_BIG)   (consts as exact 3-term fp16
                 split; PSUM stores ~3072+s, fp32 ulp 2^-12 = quantizer)
  P2 [128 rows]: zm[0:63].Eh + zh.Em - B_BIG
                 (-B is the LAST moving row: the chain rounds once at
                 2^-12 and PSUM recenters to w_q exactly; 1 zm dim dropped
                 to fit 128 rows, error ~1 quantum)
  P3 [rank-1, 512 cols/group]: + (k%16)*2^-16  (fp16 subnormals are exact;
                 chain stays small so the payload survives; PSUM now holds
                 w_q + payload for both tiles)
Stage 1: one DVE reduce_min over 16-wide k-chunks straight from PSUM.
Level 2 as v1: global min, equality bitmask dotted with 2^j,
lowest-set-bit -> chunk j*; payload of the min -> i*; code = 16*j* + i*.
Ties resolve to the smallest k, matching jnp.argmin.
"""

import sys

for _p in ("/opt/trn_rl_repo", "/root/.axon_site/_ro/trn_rl_repo"):
    if _p not in sys.path:
        sys.path.insert(0, _p)

import numpy as np

import concourse.bass as bass
import concourse.bacc as bacc
import concourse.mybir as mybir
from concourse import tile
from concourse.bass_utils import run_bass_kernel_spmd

F32 = mybir.dt.float32
FP16 = mybir.dt.float16
I32 = mybir.dt.int32

N_CORES = 8
T_TOTAL = 16
N_SLICES = T_TOTAL // N_CORES          # t-slices per core
POINTS = 128 * 128                     # points per t-slice
N_CHUNKS = 4                           # point-chunks per slice
CHUNK_PTS = POINTS // N_CHUNKS         # 4096
TILES_PER_CHUNK = CHUNK_PTS // 128     # 32
K = 256
D = 64

B_BIG = 3072.0                         # w+B in [2048,4096) -> quantum 2^-12
DELTA = 2.0 ** -16                     # payload step for i = k % 16
PAYSCALE = 2.0 ** 16

AluOp = mybir.AluOpType
Axis = mybir.AxisListType


def _build_nc():
    nc = bacc.Bacc(None, target_bir_lowering=False, debug=False)

    # A-plane: rows 0-62 zm (dim 63 dropped), 63-126 zh, 127 ones
    zpa_d = nc.declare_dram_parameter(
        "zpa", [N_SLICES, 128, POINTS], FP16, isOutput=False
    )
    # B-plane: rows 0-63 zh, 64-67 ones
    zpb_d = nc.declare_dram_parameter(
        "zpb", [N_SLICES, D + 4, POINTS], FP16, isOutput=False
    )
    mov1_d = nc.declare_dram_parameter("mov1", [D + 4, K], FP16, isOutput=False)
    mov2_d = nc.declare_dram_parameter("mov2", [128, K], FP16, isOutput=False)
    pay_d = nc.declare_dram_parameter("payrow", [1, 4 * K], FP16, isOutput=False)
    pow2_d = nc.declare_dram_parameter("pow2", [128, 16], F32, isOutput=False)
    codes_d = nc.declare_dram_parameter(
        "codes", [N_SLICES, 128, N_CHUNKS, TILES_PER_CHUNK], I32, isOutput=True
    )

    with tile.TileContext(nc) as tc:
        with (
            tc.tile_pool(name="cst", bufs=1) as cst_pool,
            tc.tile_pool(name="chunk", bufs=3) as chunk_pool,
            tc.tile_pool(name="psum", bufs=2, space="PSUM") as psum_pool,
            tc.tile_pool(name="m16", bufs=2) as m16_pool,
            tc.tile_pool(name="l2", bufs=2) as l2_pool,
            tc.tile_pool(name="codes", bufs=2) as codes_pool,
        ):
            mov1 = cst_pool.tile([D + 4, K], FP16, tag="mov1")
            mov2 = cst_pool.tile([128, K], FP16, tag="mov2")
            # payload moving tile: rows 0-126 zero, row 127 = pay pattern;
            # pairs with chA whose row 127 is ones -> shape-uniform matmul.
            paymt = cst_pool.tile([128, 4 * K], FP16, tag="paymt")
            nc.vector.memset(paymt[0:127, :], 0.0)
            payrow = paymt[127:128, 0 : 4 * K]
            pow2b = cst_pool.tile([128, 16], F32, tag="pow2")
            nc.sync.dma_start(mov1[:], mov1_d[:])
            nc.sync.dma_start(mov2[:], mov2_d[:])
            nc.sync.dma_start(payrow, pay_d[:])
            nc.sync.dma_start(pow2b[:], pow2_d[:])

            SG = 8                          # tiles per 4-bank supergroup
            T = N_CHUNKS * TILES_PER_CHUNK  # tiles per slice (128)
            for s in range(N_SLICES):
                codes_sb = codes_pool.tile([128, T], I32)
                m16 = m16_pool.tile([128, T, 16], F32)
                for c in range(N_CHUNKS):
                    chA = chunk_pool.tile([128, CHUNK_PTS], FP16, tag="chA")
                    chB = chunk_pool.tile([D + 4, CHUNK_PTS], FP16, tag="chB")
                    rng = slice(c * CHUNK_PTS, (c + 1) * CHUNK_PTS)
                    nc.sync.dma_start(chA[:], zpa_d[s, :, rng])
                    nc.sync.dma_start(chB[:], zpb_d[s, :, rng])
                    # [*, 32, 128]: [:, j, m] = point 32*m + j
                    av = chA[:].rearrange("p (n j) -> p j n", j=TILES_PER_CHUNK)
                    bv = chB[:].rearrange("p (n j) -> p j n", j=TILES_PER_CHUNK)
                    for g in range(TILES_PER_CHUNK // SG):
                        ps = psum_pool.tile([128, SG * K], F32)
                        for h in range(SG):
                            j = g * SG + h
                            reg = ps[:, K * h : K * (h + 1)]
                            # start=True resets the whole 2KB bank (2 tiles)
                            nc.tensor.matmul(
                                reg,
                                bv[:, j, :],
                                mov1[:],
                                start=(h % 2 == 0),
                                stop=False,
                                skip_group_check=True,
                            )
                            nc.tensor.matmul(
                                reg,
                                av[:, j, :],
                                mov2[:],
                                start=False,
                                stop=False,
                                skip_group_check=True,
                            )
                        # payload pass per bank (2 tiles); stationary = chA
                        # slice (row 127 = ones), moving rows 0-126 zero
                        for q in range(SG // 2):
                            nc.tensor.matmul(
                                ps[:, 512 * q : 512 * (q + 1)],
                                av[:, g * SG + 2 * q, :],
                                paymt[:, 0:512],
                                start=False,
                                stop=True,
                                skip_group_check=True,
                            )
                        t0 = c * TILES_PER_CHUNK + g * SG
                        nc.vector.tensor_reduce(
                            m16[:, t0 : t0 + SG, :],
                            ps[:].rearrange(
                                "m (t c2 i) -> m t c2 i", t=SG, c2=16, i=16
                            ),
                            axis=Axis.X,
                            op=AluOp.min,
                        )

                # ---- level 2: whole slice (128 tiles) at once ----
                vmin = l2_pool.tile([128, T], F32)
                nc.vector.tensor_reduce(vmin[:], m16[:], axis=Axis.X, op=AluOp.min)
                eq = l2_pool.tile([128, T, 16], F32)
                nc.vector.tensor_tensor(
                    eq[:],
                    m16[:],
                    vmin[:].unsqueeze(2).broadcast_to([128, T, 16]),
                    op=AluOp.is_equal,
                )
                bs = l2_pool.tile([128, T, 16], F32)
                nc.vector.tensor_tensor(
                    bs[:],
                    eq[:],
                    pow2b[:].unsqueeze(1).broadcast_to([128, T, 16]),
                    op=AluOp.mult,
                )
                b = l2_pool.tile([128, T], F32)
                nc.vector.tensor_reduce(b[:], bs[:], axis=Axis.X, op=AluOp.add)
                bi32 = l2_pool.tile([128, T], I32)
                nc.vector.tensor_copy(bi32[:], b[:])
                nbi = l2_pool.tile([128, T], I32)
                nc.vector.tensor_scalar(nbi[:], b[:], -1.0, None, AluOp.mult)
                low = l2_pool.tile([128, T], I32)
                nc.vector.tensor_tensor(low[:], bi32[:], nbi[:], op=AluOp.bitwise_and)
                lowf = l2_pool.tile([128, T], F32)
                nc.vector.tensor_copy(lowf[:], low[:])
                # j* = (float_bits(2^j) >> 23) - 127 ; jv = 16*j*
                jt = l2_pool.tile([128, T], I32)
                nc.vector.tensor_scalar(
                    jt[:], lowf[:].bitcast(I32), 23, None, AluOp.arith_shift_right
                )
                jv = l2_pool.tile([128, T], I32)
                nc.vector.tensor_scalar(
                    jv[:], jt[:], 127, 16, AluOp.subtract, AluOp.mult
                )
                # i* = (int(vmin * 2^16)) & 15
                t1 = l2_pool.tile([128, T], I32)
                nc.vector.tensor_scalar(t1[:], vmin[:], PAYSCALE, None, AluOp.mult)
                t2 = l2_pool.tile([128, T], I32)
                nc.vector.tensor_scalar(t2[:], t1[:], 15, None, AluOp.bitwise_and)
                nc.vector.tensor_tensor(codes_sb[:], jv[:], t2[:], op=AluOp.add)
                nc.sync.dma_start(
                    codes_d[s],
                    codes_sb[:].rearrange("m (c j) -> m c j", c=N_CHUNKS),
                )
    nc.compile()
    return nc


def _make_consts(emb: np.ndarray):
    e2 = (emb.astype(np.float64) ** 2).sum(axis=-1)
    E = (-2.0 * emb.T).astype(np.float32)          # [64, 256]
    Eh = E.astype(np.float16)
    Em = (E - Eh.astype(np.float32)).astype(np.float16)

    v = e2 + B_BIG
    a1 = v.astype(np.float32).astype(np.float16)
    a2 = (v - a1.astype(np.float64)).astype(np.float32).astype(np.float16)
    a3 = (v - a1.astype(np.float64) - a2.astype(np.float64)).astype(
        np.float32
    ).astype(np.float16)

    mov1 = np.zeros((D + 4, K), dtype=np.float16)
    mov1[0:D] = Eh
    mov1[D] = a1
    mov1[D + 1] = a2
    mov1[D + 2] = a3

    mov2 = np.empty((128, K), dtype=np.float16)
    mov2[0:63] = Eh[0:63]
    mov2[63:127] = Em
    mov2[127] = np.float16(-B_BIG)

    payrow = ((np.arange(4 * K) % 16).astype(np.float32) * DELTA).astype(
        np.float16
    )[None, :]
    pow2 = np.broadcast_to(
        (2.0 ** np.arange(16)).astype(np.float32), (128, 16)
    ).copy()
    return mov1, mov2, payrow, pow2


def _pack_z(zr: np.ndarray):
    """zr [S, 64, POINTS] f32 -> (zpa [S,128,POINTS], zpb [S,68,POINTS]) fp16."""
    S, _, P = zr.shape
    zh = zr.astype(np.float16)
    zm = (zr - zh.astype(np.float32)).astype(np.float16)
    zpa = np.empty((S, 128, P), dtype=np.float16)
    zpa[:, 0:63] = zm[:, 0:63]
    zpa[:, 63:127] = zh
    zpa[:, 127] = np.float16(1.0)
    zpb = np.ones((S, D + 4, P), dtype=np.float16)
    zpb[:, 0:D] = zh
    return zpa, zpb


def _run(z: np.ndarray, emb: np.ndarray, **spmd_kwargs):
    z = np.asarray(z, dtype=np.float32)
    emb = np.asarray(emb, dtype=np.float32)
    t, a, b, c = z.shape
    assert (t, a, b, c) == (16, 64, 128, 128) and emb.shape == (256, 64)

    zr = z.reshape(t, a, b * c)
    mov1, mov2, payrow, pow2 = _make_consts(emb)

    nc = _build_nc()
    in_maps = []
    for i in range(N_CORES):
        zpa, zpb = _pack_z(zr[i * N_SLICES : (i + 1) * N_SLICES])
        in_maps.append(
            {
                "zpa": zpa,
                "zpb": zpb,
                "mov1": mov1,
                "mov2": mov2,
                "payrow": payrow,
                "pow2": pow2,
            }
        )
    res = run_bass_kernel_spmd(nc, in_maps, core_ids=list(range(N_CORES)), **spmd_kwargs)

    out = np.empty((t, b * c), dtype=np.int32)
    for i in range(N_CORES):
        arr = np.asarray(res.results[i]["codes"])  # [N_SLICES, 128, N_CHUNKS, 32]
        # point p = 4096*c + 32*m + j  ->  [s, c, m, j] order is p-major
        out[i * N_SLICES : (i + 1) * N_SLICES] = (
            arr.transpose(0, 2, 1, 3).reshape(N_SLICES, b * c).astype(np.int32)
        )
    return out.reshape(t, b, c), res


def kernel(z: np.ndarray, emb: np.ndarray) -> np.ndarray:
    return _run(z, emb)[0]

